# revision 33
# baseline (speedup 1.0000x reference)
"""MultiHeadAttention Trainium2 kernel (8-core SPMD).

Problem: B=2, T=2048, C=1024, H=16 heads, D=64.
  out = softmax((q Wq^T + bq)(k Wk^T + bk)^T / sqrt(D)) (v Wv^T + bv) Wo^T + bo

Sharding: core c -> (batch b = c // 4, head-group g = c % 4).  Each core
computes 4 heads (a 256-wide slice of the projection space) of one batch
element, including its partial contribution to the row-sharded output
projection.  The host sums the 4 partial outputs per batch (bo folded
on-device into the g==0 core's partial; bv folded into bos host-side as
Wo[:, sl] @ bv[sl]).

Design (~258us vs the 508us fp32r v1):
  - activations arrive transposed AND bf16 from the host ([C, T] in
    DRAM), killing the 384 fp32 PE transposes, their weight loads, and
    96 PSUM->SBUF copies of v1.
  - every matmul is bf16 (fp32r lowers to fp32_mode=HIGH on hw = ~2x
    slower); PSUM accumulation stays fp32.  Output is stored bf16 and
    upcast on the host.
  - the kernel is paced by two hard floors: the Scalar engine's exp
    stream (16.8M elems/core, ~140us, exp runs ONLY there) and the PE
    (~165us ideal).  Everything is scheduled to keep the exp stream
    hot: minimal lead-in (K chunk 0 + Q half 0 projected first), all
    other projections + V-transposes + the first output projection
    spread as filler between attention units, AV matmuls emitted with
    a 3-5 unit skew so they never head-of-line-block the in-order PE
    queue behind a pending exp.
  - DMA: transfers share ~300GB/s/core, so issue order == priority.
    The scalar queue gets only a minimal early set and then stays
    exp-only (ring-full waits on a queue block everything behind
    them); sync carries the rest in deadline order.
  - V^T is staged like K then PE-transposed (bf16, 1 cyc/row) into
    VN tiles that carry 64 ones columns per head, so each AV matmul
    also emits the softmax denominators into psum rows 64-127 for
    free.  Normalization uses reciprocal_approx_fast (5x faster than
    DVE reciprocal; needs SBUF input - it silently corrupts reading
    PSUM) immediately after each head's last AV so its psum bank
    frees before the pass ends.
  - PSUM budget (8 banks): S tiles 2x[128,1024] + accumulators
    2x[128,1024]; K/Q/V/out projections borrow S-ring slots.
Known dead ends: fp8e4m3 DoubleRow scores work mechanically (needs the
64-partition split: PE tile bases allow only 0/32/64) but land at
2.4e-2 rel err vs the 2e-2 gate; DMA cannot read PSUM; tensor_tensor
cannot read two PSUM operands; exp+reciprocal live in different act
tables (never put reciprocal on Scalar).
"""

import numpy as np

B, T, C, H, D = 2, 2048, 1024, 16, 64
NCORES = 8
GROUPS = 4              # head-groups == cores per batch element
HG = H // GROUPS        # heads per core
DS = HG * D             # per-core projection slice width (256)
TCH = 512               # token chunk (psum bank = 512 fp32)
NTCH = T // TCH         # 4
NCC = C // 128          # 8 contraction chunks
NKT = T // 128          # 16 key tiles
SCALE = float(D) ** -0.5

_NC_CACHE = None


def _emit(ctx, tc, io):
    import concourse.bass as bass
    from concourse import mybir

    nc = tc.nc
    f32 = mybir.dt.float32
    bf16 = mybir.dt.bfloat16
    EXP = mybir.ActivationFunctionType.Exp

    persist = ctx.enter_context(tc.tile_pool(name="persist", bufs=1))

    def ptile(tag, shape, dt=f32):
        return persist.tile(shape, dt, tag=tag, name=tag)

    # --- persistent SBUF tensors (DMAs ordered for earliest stage-B) ---
    wt = {n: ptile(n, [128, NCC * DS], bf16) for n in ("wq", "wk", "wv")}
    wsb = {n: wt[n][:].rearrange("p (j c) -> p j c", j=NCC)
           for n in ("wq", "wk", "wv")}
    wot = [ptile(f"wot{dc}", [128, C], bf16) for dc in range(2)]
    bias = {n: ptile(n, [128, w]) for n, w in
            (("bqs", 2), ("bks", 2), ("bos", 8))}
    QT = [ptile(f"qt{i}", [128, T], bf16) for i in range(2)]
    KT = [ptile(f"kt{i}", [128, T], bf16) for i in range(2)]
    # V natural [key, head*(64 V + 64 ones)]: ones columns make each
    # head's AV matmul also emit the softmax denominator (rows 64-127)
    VN = [ptile(f"vn{i}", [128, 4 * 128], bf16) for i in range(NKT)]
    for kt in range(NKT):
        nc.vector.memset(
            VN[kt][:].rearrange("p (h c) -> p h c", h=4)[:, :, 64:128], 1.0)
    # warm the Exp table off the critical path
    identsb = ptile("ident", [128, 128], bf16)
    warm = ptile("warm", [1, 8])
    nc.vector.memset(warm[:], 0.0)
    nc.scalar.activation(warm[:], warm[:], EXP, scale=1.0)

    def wdma(eng, name, ap):
        eng.dma_start(ap, io[name].rearrange("(a p) o -> p (a o)", p=128)
                      if name.startswith("b") else io[name])

    # scalar queue gets ONLY the minimal early set (then stays exp-only;
    # DMA ring-full waits on a queue block everything behind them).
    # sync carries the rest in need-order; transfers share HBM bandwidth
    # so order == priority.
    def wtdma(eng, n):
        eng.dma_start(
            wt[n][:].rearrange("p (j c) -> p j c", j=NCC),
            io[n + "t"].rearrange("(j p) c -> p j c", p=128))

    xinp = ctx.enter_context(tc.tile_pool(name="xin", bufs=1))

    def xdma(eng, name, tci, slot=None):
        t_ = xinp.tile([128, NCC * TCH], bf16,
                       tag=f"{name}{tci if slot is None else slot}",
                       name=name)
        v = t_[:].rearrange("p (j t) -> p j t", j=NCC)
        eng.dma_start(
            v, io[name][:, tci * TCH:(tci + 1) * TCH].rearrange(
                "(j p) t -> p j t", p=128))
        return v

    wdma(nc.scalar, "bqs", bias["bqs"][:])
    wdma(nc.scalar, "bks", bias["bks"][:])
    nc.scalar.dma_start(identsb[:], io["ident"][:, :])
    wdma(nc.scalar, "bos", bias["bos"][:])
    xk = [xdma(nc.sync, "xkt", 0)]
    wtdma(nc.scalar, "wk")
    xq = [xdma(nc.sync, "xqt", 0)]
    wtdma(nc.scalar, "wq")
    xq.append(xdma(nc.scalar, "xqt", 1))
    xv = [xdma(nc.sync, "xvt", 0)]
    xk.append(xdma(nc.sync, "xkt", 1))
    wtdma(nc.scalar, "wv")
    xk.append(xdma(nc.sync, "xkt", 2))
    xv.append(xdma(nc.sync, "xvt", 1))
    xk.append(xdma(nc.sync, "xkt", 3))
    xv.append(xdma(nc.sync, "xvt", 2))
    xv.append(xdma(nc.sync, "xvt", 3))
    for dc in range(2):
        nc.sync.dma_start(wot[dc][:], io["wot"][dc * 128:(dc + 1) * 128, :])
    # second Q half reuses the first half's slots; the WAR wait (on
    # Qp0/Qp1 reads) sits on the by-then idle sync queue
    xq += [xdma(nc.sync, "xqt", tci, slot=tci - 2) for tci in range(2, NTCH)]

    QCW = 2 * TCH           # stage-B query chunk (1024)
    NQC = T // QCW          # 2

    def proj(pool, wname, bname, XT, xin, tci, tag="pj", cos=(0, 1)):
        # K^T / Q^T chunk [256, 512]: W-stationary matmuls + bias-add
        for co in cos:
            pj = pool.tile([128, TCH], f32, tag=tag, name="pj")
            for cc in range(NCC):
                nc.tensor.matmul(
                    pj[:],
                    lhsT=wsb[wname][:, cc, co * 128:(co + 1) * 128],
                    rhs=xin[:, cc, :],
                    start=(cc == 0), stop=(cc == NCC - 1))
            dst = XT[co][:, tci * TCH:(tci + 1) * TCH]
            if bname is None:
                nc.vector.tensor_copy(dst, pj[:])
            else:
                nc.vector.tensor_scalar_add(
                    dst, pj[:], bias[bname][:, co:co + 1])

    # V^T [256, T] staged like K, then PE-transposed into VN tiles
    VT = [ptile(f"vt{i}", [128, T], bf16) for i in range(2)]
    ident = identsb

    def vtrans(pool, kt):
        # VN[kt][keys, h*128+d] via bf16 PE transposes of VT chunks
        pv = pool.tile([128, 2 * 128], bf16, tag="s", name="pv")
        for co in range(2):
            nc.tensor.matmul(
                pv[:, co * 128:(co + 1) * 128],
                lhsT=VT[co][:, kt * 128:(kt + 1) * 128],
                rhs=ident[:],
                is_transpose=True,
                start=(co == 0), stop=(co == 1))
        nc.vector.tensor_copy(
            VN[kt][:].rearrange("p (h c) -> p h c", h=4)[:, :, 0:64],
            pv[:].rearrange("p (h d) -> p h d", h=4))

    # --- minimal lead-in: K(0), Q(0), Q(1); rest spread into stage B ---
    with tc.tile_pool(name="kpj", bufs=4, space="PSUM") as kpj:
        proj(kpj, "wk", "bks", KT, xk[0], 0)
        proj(kpj, "wq", "bqs", QT, xq[0], 0)
        proj(kpj, "wq", "bqs", QT, xq[1], 1)

    # --- stage B: attention, with projections/outproj as filler --------
    with tc.tile_pool(name="sps", bufs=2, space="PSUM") as sps, \
         tc.tile_pool(name="otps", bufs=2, space="PSUM") as otps, \
         tc.tile_pool(name="expsb", bufs=8) as expsb, \
         tc.tile_pool(name="osbp", bufs=3) as osbp, \
         tc.tile_pool(name="recsb", bufs=2) as recp, \
         tc.tile_pool(name="obp", bufs=3) as obp:

        ot_sb = {}

        def outproj_ct(qc, ct):
            # one 128-channel slice of the output projection + store
            qcols = slice(qc * QCW, (qc + 1) * QCW)
            ob = obp.tile([128, QCW], bf16, tag="ob", name="ob")
            pp = sps.tile([128, QCW], f32, tag="s", name="prj")
            for half in range(2):
                hc = slice(half * TCH, (half + 1) * TCH)
                for dc in range(2):
                    nc.tensor.matmul(
                        pp[:, hc],
                        lhsT=wot[dc][:, ct * 128:(ct + 1) * 128],
                        rhs=ot_sb[(qc, dc)][:, hc],
                        start=(dc == 0), stop=(dc == 1))
            nc.vector.tensor_scalar_add(
                ob[:], pp[:], bias["bos"][:, ct:ct + 1])
            nc.sync.dma_start(
                io["out_t"][ct * 128:(ct + 1) * 128, qcols], ob[:])

        # filler work keyed by (qc, pr, kt, hh): K/Q projections for later
        # chunks, V-natural tiles, and the first half's output projection
        extras = {}
        for kt in range(NKT):
            ex = []
            if kt % 4 == 0:
                ex.append(lambda t=kt // 4: proj(
                    sps, "wv", None, VT, xv[t], t, tag="s"))
            ex.append(lambda kt=kt: vtrans(sps, kt))
            extras[(0, 0, kt, 1)] = ex
        for i, kt in enumerate((2, 6, 10)):
            for co in range(2):
                extras.setdefault((0, 0, kt + co, 0), []).append(
                    lambda i=i, co=co: proj(
                        sps, "wk", "bks", KT, xk[i + 1], i + 1,
                        tag="s", cos=(co,)))
        for i, kt in enumerate((2, 8)):
            for co in range(2):
                extras.setdefault((0, 1, kt + 2 * co, 0), []).append(
                    lambda i=i, co=co: proj(
                        sps, "wq", "bqs", QT, xq[i + 2], i + 2,
                        tag="s", cos=(co,)))
        for ct in range(NCC):
            extras.setdefault((1, 0, 1 + 2 * ct, 0), []).append(
                lambda ct=ct: outproj_ct(0, ct))

        for qc in range(NQC):
            qcols = slice(qc * QCW, (qc + 1) * QCW)
            for pr in range(2):
                otp = [otps.tile([128, QCW], f32, tag="ot", name="ot")
                       for _ in range(2)]

                def do_av(kt, hh, es):
                    h = pr * 2 + hh
                    for half in range(2):
                        hc = slice(half * TCH, (half + 1) * TCH)
                        nc.tensor.matmul(
                            otp[hh][:, hc],
                            lhsT=VN[kt][:, h * 128:(h + 1) * 128],
                            rhs=es[:, hc],
                            start=(kt == 0), stop=(kt == NKT - 1))
                    if kt == NKT - 1:
                        # normalize this head right away so its psum
                        # frees before the pass ends
                        zsb = recp.tile([64, QCW], f32, tag="z", name="z")
                        nc.vector.tensor_copy(zsb[:], otp[hh][64:128, :])
                        rec = recp.tile([64, QCW], f32, tag="rec",
                                        name="rec")
                        nc.vector.reciprocal_approx_fast(rec[:], zsb[:])
                        nc.vector.tensor_mul(
                            osb[hh * 64:(hh + 1) * 64, :],
                            otp[hh][0:64, :], rec[:])

                osb = osbp.tile([128, QCW], bf16, tag="osb", name="osb")
                pend = []
                for kt in range(NKT):
                    for hh in range(2):
                        rows = slice(hh * 64, (hh + 1) * 64)
                        S = sps.tile([128, QCW], f32, tag="s", name="s")
                        for half in range(2):
                            hc = slice(half * TCH, (half + 1) * TCH)
                            qhc = slice(qc * QCW + half * TCH,
                                        qc * QCW + (half + 1) * TCH)
                            nc.tensor.matmul(
                                S[:, hc],
                                lhsT=KT[pr][rows, kt * 128:(kt + 1) * 128],
                                rhs=QT[pr][rows, qhc],
                                start=True, stop=True)
                        es = expsb.tile([128, QCW], bf16, tag="es",
                                        name="es")
                        nc.scalar.activation(es[:], S[:], EXP, scale=SCALE)
                        for fn in extras.pop((qc, pr, kt, hh), ()):
                            fn()
                        pend.append((kt, hh, es))
                        depth = 5 if (qc == 0 and pr == 0) else 4
                        if len(pend) > depth:
                            do_av(*pend.pop(0))
                for p_ in pend:
                    do_av(*p_)
                ot_sb[(qc, pr)] = osb
        for ct in range(NCC):
            outproj_ct(1, ct)


def build_nc(reps=1):
    from contextlib import ExitStack

    import concourse.tile as tile
    from concourse import bacc, mybir

    f32 = mybir.dt.float32
    bf16 = mybir.dt.bfloat16
    nc = bacc.Bacc("TRN2", target_bir_lowering=False, debug=False,
                   num_devices=NCORES)
    io = {}
    for name in ("xqt", "xkt", "xvt"):
        io[name] = nc.dram_tensor(name, [C, T], bf16,
                                  kind="ExternalInput").ap()
    for name in ("wqt", "wkt", "wvt"):
        io[name] = nc.dram_tensor(name, [C, DS], bf16,
                                  kind="ExternalInput").ap()
    io["wot"] = nc.dram_tensor("wot", [DS, C], bf16, kind="ExternalInput").ap()
    for name in ("bqs", "bks"):
        io[name] = nc.dram_tensor(name, [DS, 1], f32, kind="ExternalInput").ap()
    io["bos"] = nc.dram_tensor("bos", [C, 1], f32, kind="ExternalInput").ap()
    io["ident"] = nc.dram_tensor("ident", [128, 128], bf16,
                                 kind="ExternalInput").ap()
    io["out_t"] = nc.dram_tensor("out_t", [C, T], bf16,
                                 kind="ExternalOutput").ap()

    with tile.TileContext(nc) as tc:
        if reps == 1:
            with ExitStack() as ctx:
                _emit(ctx, tc, io)
        else:
            with tc.For_i(0, reps, 1):
                with ExitStack() as ctx:
                    _emit(ctx, tc, io)
    nc.compile()
    return nc


def get_nc():
    global _NC_CACHE
    if _NC_CACHE is None:
        _NC_CACHE = build_nc()
    return _NC_CACHE


def make_in_maps(q, k, v, Wq, bq, Wk, bk, Wv, bv, Wo, bo):
    import ml_dtypes
    bf16 = ml_dtypes.bfloat16

    q, k, v = (np.asarray(x, np.float32) for x in (q, k, v))
    Wq, Wk, Wv, Wo = (np.asarray(x, np.float32) for x in (Wq, Wk, Wv, Wo))
    bq, bk, bv, bo = (np.asarray(x, np.float32) for x in (bq, bk, bv, bo))
    # per-batch transposed bf16 activations, shared across the 4 cores
    xt = {}
    for b in range(B):
        xt[b] = {
            "xqt": np.ascontiguousarray(q[b].T).astype(bf16),
            "xkt": np.ascontiguousarray(k[b].T).astype(bf16),
            "xvt": np.ascontiguousarray(v[b].T).astype(bf16),
        }
    in_maps = []
    for core in range(NCORES):
        b, g = divmod(core, GROUPS)
        sl = slice(g * DS, (g + 1) * DS)
        # bv's contribution to the output is Wo[:, sl] @ bv[sl] per token
        # (attention weights sum to 1), fold it into bos.
        bos = Wo[:, sl] @ bv[sl]
        if g == 0:
            bos = bos + bo
        in_maps.append({
            **xt[b],
            "wqt": np.ascontiguousarray(Wq[sl, :].T).astype(bf16),
            "wkt": np.ascontiguousarray(Wk[sl, :].T).astype(bf16),
            "wvt": np.ascontiguousarray(Wv[sl, :].T).astype(bf16),
            "wot": np.ascontiguousarray(Wo[:, sl].T).astype(bf16),
            "bqs": np.ascontiguousarray(bq[sl].reshape(DS, 1)),
            "bks": np.ascontiguousarray(bk[sl].reshape(DS, 1)),
            "bos": np.ascontiguousarray(bos.reshape(C, 1), dtype=np.float32),
            "ident": np.eye(128, dtype=np.float32).astype(bf16),
        })
    return in_maps


def combine(results):
    out = np.zeros((B, T, C), np.float32)
    for core in range(NCORES):
        b, _ = divmod(core, GROUPS)
        out[b] += results[core]["out_t"].astype(np.float32).T
    return out


def kernel(q, k, v, Wq, bq, Wk, bk, Wv, bv, Wo, bo):
    from concourse.bass_utils import run_bass_kernel_spmd

    nc = get_nc()
    in_maps = make_in_maps(q, k, v, Wq, bq, Wk, bk, Wv, bv, Wo, bo)
    res = run_bass_kernel_spmd(nc, in_maps, core_ids=list(range(NCORES)))
    return combine(res.results)


# revision 34
# speedup vs baseline: 1.2092x; 1.2092x over previous
"""MultiHeadAttention Trainium2 kernel (8-core SPMD).

Problem: B=2, T=2048, C=1024, H=16 heads, D=64.
  out = softmax((q Wq^T + bq)(k Wk^T + bk)^T / sqrt(D)) (v Wv^T + bv) Wo^T + bo

Sharding: core c -> (batch b = c // 4, head-group g = c % 4).  Each core
computes 4 heads (a 256-wide slice of the projection space) of one batch
element, including its partial contribution to the row-sharded output
projection.  The host sums the 4 partial outputs per batch (bo folded
on-device into the g==0 core's partial; bv folded into bos host-side as
Wo[:, sl] @ bv[sl]).

Design (~258us vs the 508us fp32r v1):
  - activations arrive transposed AND bf16 from the host ([C, T] in
    DRAM), killing the 384 fp32 PE transposes, their weight loads, and
    96 PSUM->SBUF copies of v1.
  - every matmul is bf16 (fp32r lowers to fp32_mode=HIGH on hw = ~2x
    slower); PSUM accumulation stays fp32.  Output is stored bf16 and
    upcast on the host.
  - the kernel is paced by two hard floors: the Scalar engine's exp
    stream (16.8M elems/core, ~140us, exp runs ONLY there) and the PE
    (~165us ideal).  Everything is scheduled to keep the exp stream
    hot: minimal lead-in (K chunk 0 + Q half 0 projected first), all
    other projections + V-transposes + the first output projection
    spread as filler between attention units, AV matmuls emitted with
    a 3-5 unit skew so they never head-of-line-block the in-order PE
    queue behind a pending exp.
  - DMA: transfers share ~300GB/s/core, so issue order == priority.
    The scalar queue gets only a minimal early set and then stays
    exp-only (ring-full waits on a queue block everything behind
    them); sync carries the rest in deadline order.
  - V^T is staged like K then PE-transposed (bf16, 1 cyc/row) into
    VN tiles that carry 64 ones columns per head, so each AV matmul
    also emits the softmax denominators into psum rows 64-127 for
    free.  Normalization uses reciprocal_approx_fast (5x faster than
    DVE reciprocal; needs SBUF input - it silently corrupts reading
    PSUM) immediately after each head's last AV so its psum bank
    frees before the pass ends.
  - PSUM budget (8 banks): S tiles 2x[128,1024] + accumulators
    2x[128,1024]; K/Q/V/out projections borrow S-ring slots.
Known dead ends: fp8e4m3 DoubleRow scores work mechanically (needs the
64-partition split: PE tile bases allow only 0/32/64) but land at
2.4e-2 rel err vs the 2e-2 gate; DMA cannot read PSUM; tensor_tensor
cannot read two PSUM operands; exp+reciprocal live in different act
tables (never put reciprocal on Scalar).
"""

import numpy as np

B, T, C, H, D = 2, 2048, 1024, 16, 64
NCORES = 8
GROUPS = 4              # head-groups == cores per batch element
HG = H // GROUPS        # heads per core
DS = HG * D             # per-core projection slice width (256)
TCH = 512               # token chunk (psum bank = 512 fp32)
NTCH = T // TCH         # 4
NCC = C // 128          # 8 contraction chunks
NKT = T // 128          # 16 key tiles
SCALE = float(D) ** -0.5

_NC_CACHE = None


def _emit(ctx, tc, io):
    import concourse.bass as bass
    from concourse import mybir

    nc = tc.nc
    f32 = mybir.dt.float32
    bf16 = mybir.dt.bfloat16
    EXP = mybir.ActivationFunctionType.Exp

    persist = ctx.enter_context(tc.tile_pool(name="persist", bufs=1))

    def ptile(tag, shape, dt=f32):
        return persist.tile(shape, dt, tag=tag, name=tag)

    # --- persistent SBUF tensors (DMAs ordered for earliest stage-B) ---
    wt = {n: ptile(n, [128, NCC * DS], bf16) for n in ("wq", "wk", "wv")}
    wsb = {n: wt[n][:].rearrange("p (j c) -> p j c", j=NCC)
           for n in ("wq", "wk", "wv")}
    wot = [ptile(f"wot{dc}", [128, C], bf16) for dc in range(2)]
    bias = {n: ptile(n, [128, w]) for n, w in
            (("bqs", 2), ("bks", 2), ("bos", 8))}
    QT = [ptile(f"qt{i}", [128, T], bf16) for i in range(2)]
    KT = [ptile(f"kt{i}", [128, T], bf16) for i in range(2)]
    # V natural [key, head*(64 V + 64 ones)]: ones columns make each
    # head's AV matmul also emit the softmax denominator (rows 64-127)
    VN = [ptile(f"vn{i}", [128, 4 * 128], bf16) for i in range(NKT)]
    for kt in range(NKT):
        nc.vector.memset(
            VN[kt][:].rearrange("p (h c) -> p h c", h=4)[:, :, 64:128], 1.0)
    # warm the Exp table off the critical path
    identsb = ptile("ident", [128, 128], bf16)
    warm = ptile("warm", [1, 8])
    nc.vector.memset(warm[:], 0.0)
    nc.scalar.activation(warm[:], warm[:], EXP, scale=1.0)

    def wdma(eng, name, ap):
        eng.dma_start(ap, io[name].rearrange("(a p) o -> p (a o)", p=128)
                      if name.startswith("b") else io[name])

    # scalar queue gets ONLY the minimal early set (then stays exp-only;
    # DMA ring-full waits on a queue block everything behind them).
    # sync carries the rest in need-order; transfers share HBM bandwidth
    # so order == priority.
    def wtdma(eng, n):
        eng.dma_start(
            wt[n][:].rearrange("p (j c) -> p j c", j=NCC),
            io[n + "t"].rearrange("(j p) c -> p j c", p=128))

    xinp = ctx.enter_context(tc.tile_pool(name="xin", bufs=1))

    def xdma(eng, name, tci, slot=None):
        t_ = xinp.tile([128, NCC * TCH], bf16,
                       tag=f"{name}{tci if slot is None else slot}",
                       name=name)
        v = t_[:].rearrange("p (j t) -> p j t", j=NCC)
        eng.dma_start(
            v, io[name][:, tci * TCH:(tci + 1) * TCH].rearrange(
                "(j p) t -> p j t", p=128))
        return v

    wdma(nc.scalar, "bqs", bias["bqs"][:])
    wdma(nc.scalar, "bks", bias["bks"][:])
    nc.scalar.dma_start(identsb[:], io["ident"][:, :])
    wdma(nc.scalar, "bos", bias["bos"][:])
    xk = [xdma(nc.sync, "xkt", 0)]
    wtdma(nc.scalar, "wk")
    xq = [xdma(nc.sync, "xqt", 0)]
    wtdma(nc.scalar, "wq")
    xq.append(xdma(nc.scalar, "xqt", 1))
    xv = [xdma(nc.sync, "xvt", 0)]
    xk.append(xdma(nc.sync, "xkt", 1))
    wtdma(nc.scalar, "wv")
    xk.append(xdma(nc.sync, "xkt", 2))
    xv.append(xdma(nc.sync, "xvt", 1))
    xk.append(xdma(nc.sync, "xkt", 3))
    xv.append(xdma(nc.sync, "xvt", 2))
    xv.append(xdma(nc.sync, "xvt", 3))
    for dc in range(2):
        nc.sync.dma_start(wot[dc][:], io["wot"][dc * 128:(dc + 1) * 128, :])
    # second Q half reuses the first half's slots; the WAR wait (on
    # Qp0/Qp1 reads) sits on the by-then idle sync queue
    xq += [xdma(nc.sync, "xqt", tci, slot=tci - 2) for tci in range(2, NTCH)]

    QCW = 2 * TCH           # stage-B query chunk (1024)
    NQC = T // QCW          # 2

    def proj(pool, wname, bname, XT, xin, tci, tag="pj"):
        # K^T / Q^T chunk [256, 512]: W-stationary matmuls + bias-add
        for co in range(2):
            pj = pool.tile([128, TCH], f32, tag=tag, name="pj")
            for cc in range(NCC):
                nc.tensor.matmul(
                    pj[:],
                    lhsT=wsb[wname][:, cc, co * 128:(co + 1) * 128],
                    rhs=xin[:, cc, :],
                    start=(cc == 0), stop=(cc == NCC - 1))
            dst = XT[co][:, tci * TCH:(tci + 1) * TCH]
            if bname is None:
                nc.vector.tensor_copy(dst, pj[:])
            else:
                nc.vector.tensor_scalar_add(
                    dst, pj[:], bias[bname][:, co:co + 1])

    # V^T [256, T] staged like K, then PE-transposed into VN tiles
    VT = [ptile(f"vt{i}", [128, T], bf16) for i in range(2)]
    ident = identsb

    def vtrans(pool, kt):
        # VN[kt][keys, h*128+d] via bf16 PE transposes of VT chunks
        pv = pool.tile([128, 2 * 128], bf16, tag="s", name="pv")
        for co in range(2):
            nc.tensor.matmul(
                pv[:, co * 128:(co + 1) * 128],
                lhsT=VT[co][:, kt * 128:(kt + 1) * 128],
                rhs=ident[:],
                is_transpose=True,
                start=(co == 0), stop=(co == 1))
        nc.vector.tensor_copy(
            VN[kt][:].rearrange("p (h c) -> p h c", h=4)[:, :, 0:64],
            pv[:].rearrange("p (h d) -> p h d", h=4))

    # --- minimal lead-in: K(0), Q(0), Q(1); rest spread into stage B ---
    with tc.tile_pool(name="kpj", bufs=4, space="PSUM") as kpj:
        proj(kpj, "wk", "bks", KT, xk[0], 0)
        proj(kpj, "wq", "bqs", QT, xq[0], 0)
        proj(kpj, "wq", "bqs", QT, xq[1], 1)

    # --- stage B: attention, with projections/outproj as filler --------
    with tc.tile_pool(name="sps", bufs=2, space="PSUM") as sps, \
         tc.tile_pool(name="otps", bufs=2, space="PSUM") as otps, \
         tc.tile_pool(name="expsb", bufs=8) as expsb, \
         tc.tile_pool(name="osbp", bufs=3) as osbp, \
         tc.tile_pool(name="recsb", bufs=2) as recp, \
         tc.tile_pool(name="obp", bufs=3) as obp:

        ot_sb = {}

        def outproj_cg(qc, cg):
            # one 256-channel group of the output projection + store
            qcols = slice(qc * QCW, (qc + 1) * QCW)
            ob = obp.tile([128, 2 * QCW], bf16, tag="ob", name="ob")
            obv = ob[:].rearrange("p (j t) -> p j t", j=2)
            for cj in range(2):
                ct = cg * 2 + cj
                pp = sps.tile([128, QCW], f32, tag="s", name="prj")
                for half in range(2):
                    hc = slice(half * TCH, (half + 1) * TCH)
                    for dc in range(2):
                        nc.tensor.matmul(
                            pp[:, hc],
                            lhsT=wot[dc][:, ct * 128:(ct + 1) * 128],
                            rhs=ot_sb[(qc, dc)][:, hc],
                            start=(dc == 0), stop=(dc == 1))
                nc.vector.tensor_scalar_add(
                    obv[:, cj, :], pp[:], bias["bos"][:, ct:ct + 1])
            nc.sync.dma_start(
                io["out_t"][cg * 256:(cg + 1) * 256, qcols].rearrange(
                    "(j p) t -> p j t", p=128), obv)

        # filler work keyed by (qc, pr, kt, hh): K/Q projections for later
        # chunks, V-natural tiles, and the first half's output projection
        extras = {}
        for kt in range(NKT):
            ex = []
            if kt % 4 == 0:
                ex.append(lambda t=kt // 4: proj(
                    sps, "wv", None, VT, xv[t], t, tag="s"))
            ex.append(lambda kt=kt: vtrans(sps, kt))
            extras[(0, 0, kt, 1)] = ex
        for i, kt in enumerate((2, 6, 10)):
            extras.setdefault((0, 0, kt, 0), []).append(
                lambda i=i: proj(sps, "wk", "bks", KT, xk[i + 1], i + 1,
                                 tag="s"))
        for i, kt in enumerate((2, 8)):
            extras.setdefault((0, 1, kt, 0), []).append(
                lambda i=i: proj(sps, "wq", "bqs", QT, xq[i + 2], i + 2,
                                 tag="s"))
        for cg in range(4):
            extras.setdefault((1, 0, 2 + 3 * cg, 0), []).append(
                lambda cg=cg: outproj_cg(0, cg))

        for qc in range(NQC):
            qcols = slice(qc * QCW, (qc + 1) * QCW)
            for pr in range(2):
                otp = [otps.tile([128, QCW], f32, tag="ot", name="ot")
                       for _ in range(2)]

                def do_av(kt, hh, es):
                    h = pr * 2 + hh
                    for half in range(2):
                        hc = slice(half * TCH, (half + 1) * TCH)
                        nc.tensor.matmul(
                            otp[hh][:, hc],
                            lhsT=VN[kt][:, h * 128:(h + 1) * 128],
                            rhs=es[:, hc],
                            start=(kt == 0), stop=(kt == NKT - 1))
                    if kt == NKT - 1:
                        # normalize this head right away so its psum
                        # frees before the pass ends
                        zsb = recp.tile([64, QCW], f32, tag="z", name="z")
                        nc.vector.tensor_copy(zsb[:], otp[hh][64:128, :])
                        rec = recp.tile([64, QCW], f32, tag="rec",
                                        name="rec")
                        nc.vector.reciprocal_approx_fast(rec[:], zsb[:])
                        nc.vector.tensor_mul(
                            osb[hh * 64:(hh + 1) * 64, :],
                            otp[hh][0:64, :], rec[:])

                osb = osbp.tile([128, QCW], bf16, tag="osb", name="osb")
                pend = []
                for kt in range(NKT):
                    for hh in range(2):
                        rows = slice(hh * 64, (hh + 1) * 64)
                        S = sps.tile([128, QCW], f32, tag="s", name="s")
                        for half in range(2):
                            hc = slice(half * TCH, (half + 1) * TCH)
                            qhc = slice(qc * QCW + half * TCH,
                                        qc * QCW + (half + 1) * TCH)
                            nc.tensor.matmul(
                                S[:, hc],
                                lhsT=KT[pr][rows, kt * 128:(kt + 1) * 128],
                                rhs=QT[pr][rows, qhc],
                                start=True, stop=True)
                        es = expsb.tile([128, QCW], bf16, tag="es",
                                        name="es")
                        nc.scalar.activation(es[:], S[:], EXP, scale=SCALE)
                        for fn in extras.pop((qc, pr, kt, hh), ()):
                            fn()
                        pend.append((kt, hh, es))
                        depth = 5 if (qc == 0 and pr == 0) else 4
                        if len(pend) > depth:
                            do_av(*pend.pop(0))
                for p_ in pend:
                    do_av(*p_)
                ot_sb[(qc, pr)] = osb
        for cg in range(4):
            outproj_cg(1, cg)


def build_nc(reps=1):
    from contextlib import ExitStack

    import concourse.tile as tile
    from concourse import bacc, mybir

    f32 = mybir.dt.float32
    bf16 = mybir.dt.bfloat16
    nc = bacc.Bacc("TRN2", target_bir_lowering=False, debug=False,
                   num_devices=NCORES)
    io = {}
    for name in ("xqt", "xkt", "xvt"):
        io[name] = nc.dram_tensor(name, [C, T], bf16,
                                  kind="ExternalInput").ap()
    for name in ("wqt", "wkt", "wvt"):
        io[name] = nc.dram_tensor(name, [C, DS], bf16,
                                  kind="ExternalInput").ap()
    io["wot"] = nc.dram_tensor("wot", [DS, C], bf16, kind="ExternalInput").ap()
    for name in ("bqs", "bks"):
        io[name] = nc.dram_tensor(name, [DS, 1], f32, kind="ExternalInput").ap()
    io["bos"] = nc.dram_tensor("bos", [C, 1], f32, kind="ExternalInput").ap()
    io["ident"] = nc.dram_tensor("ident", [128, 128], bf16,
                                 kind="ExternalInput").ap()
    io["out_t"] = nc.dram_tensor("out_t", [C, T], bf16,
                                 kind="ExternalOutput").ap()

    with tile.TileContext(nc) as tc:
        if reps == 1:
            with ExitStack() as ctx:
                _emit(ctx, tc, io)
        else:
            with tc.For_i(0, reps, 1):
                with ExitStack() as ctx:
                    _emit(ctx, tc, io)
    nc.compile()
    return nc


def get_nc():
    global _NC_CACHE
    if _NC_CACHE is None:
        _NC_CACHE = build_nc()
    return _NC_CACHE


def make_in_maps(q, k, v, Wq, bq, Wk, bk, Wv, bv, Wo, bo):
    import ml_dtypes
    bf16 = ml_dtypes.bfloat16

    q, k, v = (np.asarray(x, np.float32) for x in (q, k, v))
    Wq, Wk, Wv, Wo = (np.asarray(x, np.float32) for x in (Wq, Wk, Wv, Wo))
    bq, bk, bv, bo = (np.asarray(x, np.float32) for x in (bq, bk, bv, bo))
    # per-batch transposed bf16 activations, shared across the 4 cores
    xt = {}
    for b in range(B):
        xt[b] = {
            "xqt": np.ascontiguousarray(q[b].T).astype(bf16),
            "xkt": np.ascontiguousarray(k[b].T).astype(bf16),
            "xvt": np.ascontiguousarray(v[b].T).astype(bf16),
        }
    in_maps = []
    for core in range(NCORES):
        b, g = divmod(core, GROUPS)
        sl = slice(g * DS, (g + 1) * DS)
        # bv's contribution to the output is Wo[:, sl] @ bv[sl] per token
        # (attention weights sum to 1), fold it into bos.
        bos = Wo[:, sl] @ bv[sl]
        if g == 0:
            bos = bos + bo
        in_maps.append({
            **xt[b],
            "wqt": np.ascontiguousarray(Wq[sl, :].T).astype(bf16),
            "wkt": np.ascontiguousarray(Wk[sl, :].T).astype(bf16),
            "wvt": np.ascontiguousarray(Wv[sl, :].T).astype(bf16),
            "wot": np.ascontiguousarray(Wo[:, sl].T).astype(bf16),
            "bqs": np.ascontiguousarray(bq[sl].reshape(DS, 1)),
            "bks": np.ascontiguousarray(bk[sl].reshape(DS, 1)),
            "bos": np.ascontiguousarray(bos.reshape(C, 1), dtype=np.float32),
            "ident": np.eye(128, dtype=np.float32).astype(bf16),
        })
    return in_maps


def combine(results):
    out = np.zeros((B, T, C), np.float32)
    for core in range(NCORES):
        b, _ = divmod(core, GROUPS)
        out[b] += results[core]["out_t"].astype(np.float32).T
    return out


def kernel(q, k, v, Wq, bq, Wk, bk, Wv, bv, Wo, bo):
    from concourse.bass_utils import run_bass_kernel_spmd

    nc = get_nc()
    in_maps = make_in_maps(q, k, v, Wq, bq, Wk, bk, Wv, bv, Wo, bo)
    res = run_bass_kernel_spmd(nc, in_maps, core_ids=list(range(NCORES)))
    return combine(res.results)
